# revision 1
# baseline (speedup 1.0000x reference)
"""Transformer block (LN->per-head QKV attention->out-proj->LN->FFN, residuals)
on 8 Trainium2 NeuronCores.

Sharding: data-parallel over batch (4) x query-half (2) = 8 cores. Core c
owns batch b = c//2 and query tokens [half*512, half*512+512), half = c%2.
Each core runs LN1 + K/V projections over its batch's full 1024-token
sequence (duplicated with its pair core - cheaper than communicating), and
Q/attention/out-proj/FFN for its own 512 query tokens. The full output is
assembled on host from the 8 [512, 768] shards. All matmuls run in fp32r
(full-rate TF32-class mode; fp32 storage).

Device program details:
- LayerNorm affines are folded into adjacent weights host-side, so on-device
  LN is a pure (x - mu) * rsqrt(var + eps) tensor_scalar.
- x_b is rotated host-side so the core's query tokens lead; attention has no
  mask and sums over all keys, so key/value order is irrelevant, which lets
  Q^T be a plain slice of the transposed activations h0T.
- Scores are computed transposed ([key, query]) so softmax needs no
  transposes: exp() runs PSUM->SBUF on the scalar engine (scores ~ N(0,1),
  so no max-subtraction is needed for stability), the denominator comes from
  a ones-column matmul, and normalization is folded into the PSUM->SBUF copy
  of ctx^T against a PE-broadcast reciprocal row.
- The walrus build here rejects instructions carrying more than one sync
  wait; split_waits() hoists extras onto same-engine NOPs post-scheduling.
"""

from contextlib import ExitStack

import numpy as np

import concourse.bass as bass
import concourse.mybir as mybir
import concourse.tile as tile
from concourse.bass_utils import run_bass_kernel_spmd
from concourse.masks import make_identity

f32 = mybir.dt.float32
f32r = mybir.dt.float32r
AF = mybir.ActivationFunctionType
ALU = mybir.AluOpType
EPS = 1e-5
P = 128

CFG = dict(B=4, S=1024, D=768, H=12, DH=768, DF=3072, n_cores=8)


def _splits(total, chunk):
    return [(s, min(chunk, total - s)) for s in range(0, total, chunk)]


def _split_waits(nc, max_waits=1):
    skip = (
        mybir.InstCollectiveCompute, mybir.InstEventSemaphore,
        mybir.InstCompareAndBranch, mybir.InstIndirectBranch,
        mybir.InstBranchHint,
    )
    for f in nc.m.functions:
        for bb in f.blocks:
            out = []
            for inst in bb.instructions:
                si = inst.sync_info
                if (si is not None and si.on_wait and len(si.on_wait) > max_waits
                        and not isinstance(inst, skip)
                        and getattr(inst, "engine", None) is not None):
                    waits = list(si.on_wait)
                    extra, keep = waits[:-max_waits], waits[-max_waits:]
                    for j, w in enumerate(extra):
                        nop = mybir.InstNoOp(name=f"{inst.name}-wsplit{j}")
                        nop.engine = inst.engine
                        nop.sync_info = mybir.SyncInfo(on_wait=[w], on_update=[])
                        out.append(nop)
                    inst.sync_info = mybir.SyncInfo(
                        on_wait=keep, on_update=list(si.on_update or []))
                out.append(inst)
            bb.instructions = out
    return nc


def _ln_stats(nc, pool, x_ap, d, eps_tile, name):
    import math
    fmax = math.gcd(nc.vector.BN_STATS_FMAX, d)
    nsg = d // fmax
    stats = pool.tile([P, nsg, nc.vector.BN_STATS_DIM], f32,
                      tag=f"stats_{name}", name=f"stats_{name}")
    xg = x_ap.rearrange("p (g f) -> p g f", g=nsg)
    for sg in range(nsg):
        nc.vector.bn_stats(out=stats[:, sg, :], in_=xg[:, sg, :])
    mv = pool.tile([P, nc.vector.BN_AGGR_DIM], f32, tag=f"mv_{name}",
                   name=f"mv_{name}")
    nc.vector.bn_aggr(out=mv, in_=stats)
    r = pool.tile([P, 1], f32, tag=f"r_{name}", name=f"r_{name}")
    nc.scalar.activation(out=r, in_=mv[:, 1:2], func=AF.Sqrt, bias=eps_tile,
                         scale=1.0)
    nc.vector.reciprocal(out=r, in_=r)
    return mv[:, 0:1], r


def _build(cfg, use_bv, use_b2):
    B, S, D, H, DH, DF = (cfg[k] for k in ("B", "S", "D", "H", "DH", "DF"))
    n_cores = cfg["n_cores"]
    cpb = n_cores // B
    Sq = S // cpb
    nd, ne, nt, nf, nsq = D // P, DH // P, S // P, DF // P, Sq // P
    scale = float(DH) ** -0.5

    nc = bass.Bass()
    x_b = nc.dram_tensor("x_b", [S, D], f32, kind="ExternalInput")
    xres = nc.dram_tensor("xres", [Sq, D], f32, kind="ExternalInput")
    wq_d = nc.dram_tensor("wq", [H * D, DH], f32r, kind="ExternalInput")
    wk_d = nc.dram_tensor("wk", [H * D, DH], f32r, kind="ExternalInput")
    wv_d = nc.dram_tensor("wv", [H * D, DH], f32r, kind="ExternalInput")
    bq_d = nc.dram_tensor("bq2", [H, DH], f32, kind="ExternalInput")
    bk_d = nc.dram_tensor("bk2", [H, DH], f32, kind="ExternalInput")
    bv_d = nc.dram_tensor("bv2", [1, H, DH], f32r, kind="ExternalInput")
    wo_d = nc.dram_tensor("wo", [H * DH, D], f32r, kind="ExternalInput")
    w1_d = nc.dram_tensor("w1", [D, DF], f32r, kind="ExternalInput")
    b1_d = nc.dram_tensor("b1c", [DF], f32, kind="ExternalInput")
    w2_d = nc.dram_tensor("w2", [DF, D], f32r, kind="ExternalInput")
    b2_d = nc.dram_tensor("b2r", [1, D], f32r, kind="ExternalInput")
    y_d = nc.dram_tensor("y", [Sq, D], f32, kind="ExternalOutput")

    with tile.TileContext(nc) as tc, ExitStack() as ctx:
        singles = ctx.enter_context(tc.tile_pool(name="singles", bufs=1))
        sm = ctx.enter_context(tc.tile_pool(name="sm", bufs=2))
        psmm = ctx.enter_context(tc.tile_pool(name="psmm", bufs=3, space="PSUM"))
        pss = ctx.enter_context(tc.tile_pool(name="pss", bufs=2, space="PSUM"))
        psc = ctx.enter_context(tc.tile_pool(name="psc", bufs=2, space="PSUM"))
        psdb = ctx.enter_context(tc.tile_pool(name="psdb", bufs=1, space="PSUM"))

        ident = singles.tile([P, P], f32)
        make_identity(nc, ident)
        eps_t = singles.tile([P, 1], f32)
        nc.vector.memset(eps_t, EPS)
        ones_f = singles.tile([P, 1], f32)
        nc.vector.memset(ones_f, 1.0)
        onesr_col = singles.tile([P, 1], f32r)
        nc.vector.tensor_copy(onesr_col, ones_f)
        ones_fr = singles.tile([1, P], f32)
        nc.vector.memset(ones_fr, 1.0)
        onesr_row = singles.tile([1, P], f32r)
        nc.vector.tensor_copy(onesr_row, ones_fr)

        bq_c = singles.tile([P, H, ne], f32)
        nc.sync.dma_start(out=bq_c, in_=bq_d.rearrange("h (e p) -> p h e", p=P))
        bk_c = singles.tile([P, H, ne], f32)
        nc.sync.dma_start(out=bk_c, in_=bk_d.rearrange("h (e p) -> p h e", p=P))
        b1_c = singles.tile([P, nf], f32)
        nc.sync.dma_start(out=b1_c, in_=b1_d.rearrange("(f p) -> p f", p=P))
        if use_bv:
            bv_r = singles.tile([1, H, DH], f32r)
            nc.sync.dma_start(out=bv_r, in_=bv_d[0:1])
        if use_b2:
            b2_r = singles.tile([1, D], f32r)
            nc.sync.dma_start(out=b2_r, in_=b2_d[0:1])

        acc = singles.tile([P, nsq, D], f32)
        nc.sync.dma_start(out=acc, in_=xres.rearrange("(t p) d -> p t d", p=P))

        # phase 1: LN1 + PE transposes -> h0T [D, S] (queries are [:, :Sq])
        h0_pool = tc.tile_pool(name="h0_pool", bufs=1)
        h0p = h0_pool.__enter__()
        h0T = h0p.tile([P, nd, S], f32r)
        with tc.tile_pool(name="ph1", bufs=2) as ph1:
            src_t = x_b.rearrange("(t p) d -> p t d", p=P)
            for st in range(nt):
                xt = ph1.tile([P, D], f32, tag="xt", name="xt")
                nc.sync.dma_start(out=xt, in_=src_t[:, st])
                mu, r = _ln_stats(nc, ph1, xt, D, eps_t, "ln1")
                h0 = ph1.tile([P, D], f32, tag="h0", name="h0")
                nc.vector.tensor_scalar(out=h0, in0=xt, scalar1=mu, scalar2=r,
                                        op0=ALU.subtract, op1=ALU.mult)
                for dch in range(nd):
                    pt = psmm.tile([P, 512], f32, tag="ps_mm", name="ps_t")
                    nc.tensor.transpose(pt[:, :P], h0[:, dch * P:(dch + 1) * P],
                                        ident)
                    nc.vector.tensor_copy(h0T[:, dch, st * P:(st + 1) * P],
                                          pt[:, :P])

        # phase 2: per-head QKV projections, attention, out-proj into acc
        head_stack = ExitStack()
        wt_pool = head_stack.enter_context(tc.tile_pool(name="wt", bufs=3))
        hp = head_stack.enter_context(tc.tile_pool(name="hp", bufs=1))
        for h in range(H):
            wqr = wt_pool.tile([P, nd, DH], f32r, tag="wt", name="wqr")
            nc.sync.dma_start(
                out=wqr, in_=wq_d.rearrange("(h d p) e -> h p d e", p=P, d=nd)[h])
            wkr = wt_pool.tile([P, nd, DH], f32r, tag="wt", name="wkr")
            nc.sync.dma_start(
                out=wkr, in_=wk_d.rearrange("(h d p) e -> h p d e", p=P, d=nd)[h])
            wvr = wt_pool.tile([P, nd, DH], f32r, tag="wt", name="wvr")
            nc.sync.dma_start(
                out=wvr, in_=wv_d.rearrange("(h d p) e -> h p d e", p=P, d=nd)[h])
            wor = wt_pool.tile([P, ne, D], f32r, tag="wt", name="wor")
            nc.sync.dma_start(
                out=wor, in_=wo_d.rearrange("(h e p) d -> h p e d", p=P, e=ne)[h])

            qT = hp.tile([P, ne, Sq], f32r, tag="qT", name="qT")
            kT = hp.tile([P, ne, S], f32r, tag="kT", name="kT")
            vv = hp.tile([P, nt, DH], f32r, tag="vv", name="vv")
            for et in range(ne):
                for s0, sn in _splits(Sq, 512):
                    pq = psmm.tile([P, 512], f32, tag="ps_mm", name="ps_q")
                    for dch in range(nd):
                        nc.tensor.matmul(pq[:, :sn],
                                         wqr[:, dch, et * P:(et + 1) * P],
                                         h0T[:, dch, s0:s0 + sn],
                                         start=(dch == 0), stop=(dch == nd - 1))
                    nc.scalar.activation(out=qT[:, et, s0:s0 + sn], in_=pq[:, :sn],
                                         func=AF.Identity,
                                         bias=bq_c[:, h, et:et + 1], scale=1.0)
                for s0, sn in _splits(S, 512):
                    pk = psmm.tile([P, 512], f32, tag="ps_mm", name="ps_k")
                    for dch in range(nd):
                        nc.tensor.matmul(pk[:, :sn],
                                         wkr[:, dch, et * P:(et + 1) * P],
                                         h0T[:, dch, s0:s0 + sn],
                                         start=(dch == 0), stop=(dch == nd - 1))
                    nc.scalar.activation(out=kT[:, et, s0:s0 + sn], in_=pk[:, :sn],
                                         func=AF.Identity,
                                         bias=bk_c[:, h, et:et + 1], scale=1.0)
            for tt in range(nt):
                for e0, en in _splits(DH, 512):
                    pv = psmm.tile([P, 512], f32, tag="ps_mm", name="ps_v")
                    nmm = nd + (1 if use_bv else 0)
                    for dch in range(nd):
                        nc.tensor.matmul(pv[:, :en],
                                         h0T[:, dch, tt * P:(tt + 1) * P],
                                         wvr[:, dch, e0:e0 + en],
                                         start=(dch == 0), stop=(dch == nmm - 1))
                    if use_bv:
                        nc.tensor.matmul(pv[:, :en], onesr_row,
                                         bv_r[0:1, h, e0:e0 + en],
                                         start=False, stop=True)
                    nc.scalar.activation(out=vv[:, tt, e0:e0 + en], in_=pv[:, :en],
                                         func=AF.Copy)

            # scores^T -> exp -> pT; unnormalized, no max-subtraction needed
            pT = hp.tile([P, nt, Sq], f32r, tag="pT", name="pT")
            for tt in range(nt):
                ps_ = pss.tile([P, Sq], f32, tag="ps_s", name="ps_s")
                for et in range(ne):
                    nc.tensor.matmul(ps_, kT[:, et, tt * P:(tt + 1) * P],
                                     qT[:, et, :],
                                     start=(et == 0), stop=(et == ne - 1))
                nc.scalar.activation(out=pT[:, tt, :], in_=ps_, func=AF.Exp,
                                     scale=scale)

            pd_ = psdb.tile([1, Sq], f32, tag="ps_db", name="ps_d")
            for tt in range(nt):
                nc.tensor.matmul(pd_, onesr_col, pT[:, tt, :],
                                 start=(tt == 0), stop=(tt == nt - 1))
            recip = sm.tile([1, Sq], f32, tag="recip", name="recip")
            nc.vector.reciprocal(out=recip, in_=pd_)
            recipr = sm.tile([1, Sq], f32r, tag="recipr", name="recipr")
            nc.vector.tensor_copy(recipr, recip)
            pb_ = psdb.tile([P, Sq], f32, tag="ps_db", name="ps_b")
            nc.tensor.matmul(pb_, onesr_row, recipr, start=True, stop=True)
            rb = sm.tile([P, Sq], f32, tag="rb", name="rb")
            nc.vector.tensor_copy(rb, pb_)

            ctxnT = hp.tile([P, ne, Sq], f32r, tag="ctxnT", name="ctxnT")
            for et in range(ne):
                pc_ = psc.tile([P, Sq], f32, tag="ps_c", name="ps_c")
                for tt in range(nt):
                    nc.tensor.matmul(pc_, vv[:, tt, et * P:(et + 1) * P],
                                     pT[:, tt, :],
                                     start=(tt == 0), stop=(tt == nt - 1))
                nc.vector.tensor_tensor(out=ctxnT[:, et, :], in0=pc_, in1=rb,
                                        op=ALU.mult)

            for sqt in range(nsq):
                for d0, dn in _splits(D, 512):
                    po = psmm.tile([P, 512], f32, tag="ps_mm", name="ps_o")
                    for et in range(ne):
                        nc.tensor.matmul(po[:, :dn],
                                         ctxnT[:, et, sqt * P:(sqt + 1) * P],
                                         wor[:, et, d0:d0 + dn],
                                         start=(et == 0), stop=(et == ne - 1))
                    nc.vector.tensor_add(acc[:, sqt, d0:d0 + dn],
                                         acc[:, sqt, d0:d0 + dn], po[:, :dn])

        # phase 3: LN2 + FFN + residual
        head_stack.close()
        h0_pool.__exit__(None, None, None)
        with tc.tile_pool(name="ph3", bufs=2) as ph3, \
             tc.tile_pool(name="ffn_a", bufs=1) as ffn_a:
            h2T = ffn_a.tile([P, nd, Sq], f32r)
            for sqt in range(nsq):
                mu2, r2 = _ln_stats(nc, ph3, acc[:, sqt, :], D, eps_t, "ln2")
                h2 = ph3.tile([P, D], f32, tag="h2", name="h2")
                nc.vector.tensor_scalar(out=h2, in0=acc[:, sqt, :], scalar1=mu2,
                                        scalar2=r2, op0=ALU.subtract,
                                        op1=ALU.mult)
                for dch in range(nd):
                    pt2 = psmm.tile([P, 512], f32, tag="ps_mm", name="ps_t2")
                    nc.tensor.transpose(pt2[:, :P], h2[:, dch * P:(dch + 1) * P],
                                        ident)
                    nc.vector.tensor_copy(h2T[:, dch, sqt * P:(sqt + 1) * P],
                                          pt2[:, :P])

            relu1T = ffn_a.tile([P, nf, Sq], f32r)
            with tc.tile_pool(name="ffn_w1", bufs=1) as ffn_w1:
                w1r = ffn_w1.tile([P, nd, DF], f32r, tag="w1r", name="w1r")
                nc.sync.dma_start(out=w1r,
                                  in_=w1_d.rearrange("(d p) f -> p d f", p=P))
                for ft in range(nf):
                    pf = pss.tile([P, Sq], f32, tag="ps_s", name="ps_f")
                    for dch in range(nd):
                        nc.tensor.matmul(pf,
                                         w1r[:, dch, ft * P:(ft + 1) * P],
                                         h2T[:, dch, :],
                                         start=(dch == 0), stop=(dch == nd - 1))
                    nc.scalar.activation(out=relu1T[:, ft, :], in_=pf,
                                         func=AF.Relu, bias=b1_c[:, ft:ft + 1],
                                         scale=1.0)

            ffn_w2 = ExitStack()
            w2p = ffn_w2.enter_context(tc.tile_pool(name="ffn_w2", bufs=1))
            w2r = w2p.tile([P, nf, D], f32r, tag="w2r", name="w2r")
            nc.sync.dma_start(out=w2r, in_=w2_d.rearrange("(f p) d -> p f d", p=P))
            for sqt in range(nsq):
                for d0, dn in _splits(D, 512):
                    pff = psc.tile([P, Sq], f32, tag="ps_c", name="ps_ff")
                    nmm = nf + (1 if use_b2 else 0)
                    for ft in range(nf):
                        nc.tensor.matmul(pff[:, :dn],
                                         relu1T[:, ft, sqt * P:(sqt + 1) * P],
                                         w2r[:, ft, d0:d0 + dn],
                                         start=(ft == 0), stop=(ft == nmm - 1))
                    if use_b2:
                        nc.tensor.matmul(pff[:, :dn], onesr_row,
                                         b2_r[0:1, d0:d0 + dn],
                                         start=False, stop=True)
                    yt = ph3.tile([P, 512], f32, tag="yt", name="yt")
                    nc.vector.tensor_add(yt[:, :dn], acc[:, sqt, d0:d0 + dn],
                                         pff[:, :dn])
                    nc.sync.dma_start(
                        out=y_d.rearrange("(t p) d -> p t d", p=P)[:, sqt,
                                                                   d0:d0 + dn],
                        in_=yt[:, :dn])
            ffn_w2.close()

    return _split_waits(nc)


def _prep_host(cfg, inputs):
    B, S, D, H, DH, DF = (cfg[k] for k in ("B", "S", "D", "H", "DH", "DF"))
    n_cores = cfg["n_cores"]
    cpb = n_cores // B
    Sq = S // cpb
    ii = {k: np.asarray(v, dtype=np.float32) for k, v in inputs.items()}
    x = ii["x"]
    g1, be1, g2, be2 = ii["g1"], ii["be1"], ii["g2"], ii["be2"]

    wq_eff = (ii["Wq"] * g1[None, :, None]).reshape(H * D, DH)
    wk_eff = (ii["Wk"] * g1[None, :, None]).reshape(H * D, DH)
    wv_eff = (ii["Wv"] * g1[None, :, None]).reshape(H * D, DH)
    bq_eff = ii["bq"] + np.einsum("d,hde->he", be1, ii["Wq"])
    bk_eff = ii["bk"] + np.einsum("d,hde->he", be1, ii["Wk"])
    bv_eff = (ii["bv"] + np.einsum("d,hde->he", be1, ii["Wv"]))[None]
    w1_eff = ii["W1"] * g2[:, None]
    b1_eff = ii["b1"] + be2 @ ii["W1"]
    b2_eff = ii["b2"][None]

    use_bv = bool(np.any(bv_eff != 0))
    use_b2 = bool(np.any(b2_eff != 0))

    shared = dict(wq=np.ascontiguousarray(wq_eff), wk=np.ascontiguousarray(wk_eff),
                  wv=np.ascontiguousarray(wv_eff), bq2=bq_eff, bk2=bk_eff,
                  bv2=np.ascontiguousarray(bv_eff), wo=ii["Wo"], w1=w1_eff,
                  b1c=b1_eff, w2=ii["W2"], b2r=np.ascontiguousarray(b2_eff))
    in_maps = []
    for c in range(n_cores):
        b, half = c // cpb, c % cpb
        xq = x[b, half * Sq:(half + 1) * Sq]
        m = dict(shared)
        m["x_b"] = np.ascontiguousarray(np.roll(x[b], -half * Sq, axis=0))
        m["xres"] = np.ascontiguousarray(xq + ii["bo"][None, :])
        in_maps.append(m)
    return in_maps, use_bv, use_b2


_NC_CACHE = {}


def kernel(**inputs) -> np.ndarray:
    cfg = CFG
    in_maps, use_bv, use_b2 = _prep_host(cfg, inputs)
    key = (use_bv, use_b2)
    if key not in _NC_CACHE:
        _NC_CACHE[key] = _build(cfg, use_bv, use_b2)
    nc = _NC_CACHE[key]
    res = run_bass_kernel_spmd(nc, in_maps, list(range(cfg["n_cores"])))

    B, S, D = cfg["B"], cfg["S"], cfg["D"]
    cpb = cfg["n_cores"] // B
    Sq = S // cpb
    out = np.empty((B, S, D), np.float32)
    for c in range(cfg["n_cores"]):
        b, half = c // cpb, c % cpb
        out[b, half * Sq:(half + 1) * Sq] = res.results[c]["y"]
    return out
